# revision 32
# baseline (speedup 1.0000x reference)
"""Trainium2 Bass kernel for nn_ModelMamba (4-layer Mamba, B=8, L=2048).

Sharding: tensor-parallel pairs. Cores (2w, 2w+1) form worker w; each
worker processes 2 sequences packed back-to-back on the time axis, and
each core of the pair owns half of d_inner (4 of 8 channel chunks) for
the selective scan / dt / gate / out_proj. The xc half of in_proj +
conv + x_proj are computed redundantly on both cores (PE has slack) so
the only collective is a 2-way AllReduce of out_proj partials per
(layer, chunk).

Scan phase (the DVE bottleneck) is restructured for minimum vector-engine
cycles:
  - per (n-group j of 4 states, channel-chunk dl): ONE concatenated
    tensor_tensor_scan over [128, 4*CB] (segments chained with da=0
    boundary resets + carry injection into xs), ONE batched xs multiply
    (dtup broadcast via stride-0 AP x 4-row-broadcast B tile), ONE
    batched g multiply.
  - y = sum_n g_n and the u*Dp skip run on the TensorEngine as
    identity / diag(Dp) matmul accumulations into PSUM; the gate reads
    PSUM directly.
  - a stage is processed in two dl-pair passes so 2 PSUM accumulators +
    front-matmul pools fit in 8 PSUM banks; B/C broadcast tiles are
    re-DMAed per pass (DMA has slack) with a one-group-ahead prefetch
    chain, 4 row-DMAs per tile for queue parallelism.

All per-core variation lives in the DATA, not the program.
Host side: embedding lookup, packing, weight slicing, final head.
"""
import math
import os
import sys

for _p in ("/opt/trn_rl_repo", "/root/.axon_site/_ro/trn_rl_repo"):
    if os.path.isdir(_p) and _p not in sys.path:
        sys.path.append(_p)

import numpy as np
import ml_dtypes

import concourse.bacc as bacc
import concourse.bass as bass
import concourse.tile as tile
import concourse.mybir as mybir
from concourse.bass_utils import run_bass_kernel_spmd

F32 = mybir.dt.float32
BF16 = mybir.dt.bfloat16
ALU = mybir.AluOpType
AF = mybir.ActivationFunctionType
BF = ml_dtypes.bfloat16

B, L = 8, 2048
D_MODEL, D_INNER, D_STATE, D_CONV, NL = 512, 1024, 16, 4, 4
DT_RANK = 32
KC = D_MODEL // 128          # 4 k-chunks of d_model
DCF = D_INNER // 128         # 8 full d_inner chunks (u)
DCL = 4                      # local d_inner chunks per core
MCZ = 4                      # local z m-chunks
NG = 4                       # states per scan group
MASK_HUGE = 30000.0
RG = [[0, 1], [2, 3], [4, 5], [6, 7]]

_CACHE = {}


def _plan(lens):
    """Pack 8 sequences into 4 workers; return (T, chunks, pairs, offs)."""
    pl = [max(16, int(math.ceil((int(l) + 4) / 16.0) * 16)) for l in lens]
    order = sorted(range(B), key=lambda i: -pl[i])
    pairs = [(order[i], order[7 - i]) for i in range(4)]
    offs = {}
    tw = []
    for a, b in pairs:
        offs[a] = 0
        offs[b] = pl[a]
        tw.append(pl[a] + pl[b])
    T = max(tw)
    nch = max(1, int(math.ceil(T / 1024.0)))
    base = int(math.ceil(T / nch / 16.0) * 16)
    T = base * nch              # pad worker timeline to equal chunks
    chunks = [base] * nch
    return T, tuple(chunks), pairs, offs


def _qsplit(cl):
    qs = [512] * (cl // 512)
    if cl % 512:
        qs.append(cl % 512)
    return qs


def _bcast_row(src):
    """DRAM row slice -> [128, n] broadcast AP."""
    return bass.AP(tensor=src.tensor, offset=src.offset,
                   ap=[[0, 128]] + [list(x) for x in src.ap])


def _sap(t, offset, ap):
    """Custom strided AP view on tile t (partition dim kept)."""
    return bass.AP(tensor=t.tensor, offset=t.offset + offset,
                   ap=[list(t.ap[0])] + [list(x) for x in ap])


def _build_nc(T, chunks):
    nch = len(chunks)
    nstg = NL * nch
    stages = [(li, ci) for li in range(NL) for ci in range(nch)]
    cstart = [sum(chunks[:i]) for i in range(nch)]

    nc = bacc.Bacc(None, target_bir_lowering=False, num_devices=8)

    x0 = nc.dram_tensor("x0", [D_MODEL, T], BF16, kind="ExternalInput")
    x4 = nc.dram_tensor("x4", [D_MODEL, T], BF16, kind="ExternalOutput")
    mask_d = nc.dram_tensor("maskrow", [1, T], BF16, kind="ExternalInput")

    wix_d = nc.dram_tensor("wix", [NL, D_MODEL, D_INNER], BF16, kind="ExternalInput")
    wiz_d = nc.dram_tensor("wiz", [NL, D_MODEL, 512], BF16, kind="ExternalInput")
    convd_d = nc.dram_tensor("convd", [NL, DCF, D_CONV, 128, 128], BF16,
                             kind="ExternalInput")
    wxT_d = nc.dram_tensor("wxT", [NL, D_INNER, 64], BF16, kind="ExternalInput")
    wdtT_d = nc.dram_tensor("wdtT", [NL, DT_RANK, 512], BF16, kind="ExternalInput")
    woT_d = nc.dram_tensor("woT", [NL, 512, D_MODEL], BF16, kind="ExternalInput")
    bdt_d = nc.dram_tensor("bdt", [NL, 128, DCL], F32, kind="ExternalInput")
    cb_d = nc.dram_tensor("cb", [NL, 128, DCF], F32, kind="ExternalInput")
    acol_d = nc.dram_tensor("acol", [NL, 128, DCL * D_STATE], F32,
                            kind="ExternalInput")
    idd_d = nc.dram_tensor("ident", [128, 128], BF16, kind="ExternalInput")
    dpd_d = nc.dram_tensor("dpdiag", [NL, DCL, 128, 128], BF16,
                           kind="ExternalInput")
    bcst_d = nc.dram_tensor("bcst", [nstg, 32, chunks[0]], BF16, kind="Internal")
    ccin_d = [nc.dram_tensor(f"ccin{s}", [D_MODEL, chunks[s % nch]], BF16)
              for s in range(nstg)]
    ccout_d = [nc.dram_tensor(f"ccout{s}", [D_MODEL, chunks[s % nch]], BF16)
               for s in range(nstg)]

    CB = chunks[0]
    assert all(c == CB for c in chunks)
    CB4 = NG * CB

    with tile.TileContext(nc) as tc:
        with tc.tile_pool(name="wpool", bufs=1) as wp, \
             tc.tile_pool(name="planes", bufs=1) as pp, \
             tc.tile_pool(name="stream", bufs=1) as sp, \
             tc.tile_pool(name="psum", bufs=1, space="PSUM") as qp:

            # persistent tiles -------------------------------------------------
            xt = [pp.tile([128, T], BF16, tag=f"xt{k}", bufs=1, name=f"xt{k}")
                  for k in range(KC)]
            htile = pp.tile([128, D_STATE * DCL], F32, tag="hstate", bufs=1,
                            name="hstate")
            ident = pp.tile([128, 128], BF16, tag="ident", bufs=1, name="ident")
            nc.sync.dma_start(out=ident, in_=idd_d[:, :])

            # weight tile getters (tag ring, bufs=1 -> WAR-ordered reload)
            def w_wix():
                return [wp.tile([128, D_INNER], BF16, tag=f"wix{k}", bufs=1,
                                name=f"wix{k}") for k in range(KC)]

            def w_wiz():
                return [wp.tile([128, 512], BF16, tag=f"wiz{k}", bufs=1,
                                name=f"wiz{k}") for k in range(KC)]

            def w_convd():
                return wp.tile([128, DCF * D_CONV * 128], BF16, tag="convd",
                               bufs=1, name="convd")

            def w_wxT():
                return [wp.tile([128, 64], BF16, tag=f"wxT{k}", bufs=1,
                                name=f"wxT{k}") for k in range(DCF)]

            def w_wdtT():
                return wp.tile([DT_RANK, 512], BF16, tag="wdtT", bufs=1,
                               name="wdtT")

            def w_woT():
                # bufs=2: out_proj of layer li is emitted after layer li+1's
                # weight load; two live versions required.
                return [wp.tile([128, D_MODEL], BF16, tag=f"woT{k}", bufs=2,
                                name=f"woT{k}") for k in range(DCL)]

            def w_dpd():
                return wp.tile([128, DCL * 128], BF16, tag="dpd", bufs=2,
                               name="dpd")

            def w_cols(tag):
                n = {"bdt": DCL, "cb": DCF, "acol": DCL * D_STATE}[tag]
                bufs = 2 if tag == "acol" else 1
                return wp.tile([128, n], F32, tag=tag, bufs=bufs, name=tag)

            # per-layer weight tile handles, filled by emit_weight_load
            W = {}

            def emit_weight_load(li):
                # weight DMAs ride the idle gpsimd queue: on the in-order SP
                # queue their WAR-blocked triggers would stall the
                # latency-critical B/C prefetches behind them
                for tag, src in (("bdt", bdt_d), ("cb", cb_d), ("acol", acol_d)):
                    W[tag] = w_cols(tag)
                    # tiny, latency-critical (first exps of the layer wait on
                    # acol): SP queue is clear now that big weights are on
                    # gpsimd, so land these immediately
                    nc.sync.dma_start(out=W[tag], in_=src[li])
                W["wix"] = w_wix()
                for k in range(KC):
                    nc.gpsimd.dma_start(out=W["wix"][k],
                                        in_=wix_d[li, k * 128:(k + 1) * 128, :])
                W["wiz"] = w_wiz()
                for k in range(KC):
                    nc.gpsimd.dma_start(out=W["wiz"][k],
                                        in_=wiz_d[li, k * 128:(k + 1) * 128, :])
                W["convd"] = w_convd()
                for d in range(DCF):
                    for k in range(D_CONV):
                        nc.gpsimd.dma_start(
                            out=W["convd"][:, (d * D_CONV + k) * 128:
                                           (d * D_CONV + k + 1) * 128],
                            in_=convd_d[li, d, k])
                W["wxT"] = w_wxT()
                for k in range(DCF):
                    nc.gpsimd.dma_start(out=W["wxT"][k],
                                        in_=wxT_d[li, k * 128:(k + 1) * 128, :])
                W["wdtT"] = w_wdtT()
                nc.gpsimd.dma_start(out=W["wdtT"], in_=wdtT_d[li])
                W["woT"] = w_woT()
                for k in range(DCL):
                    nc.gpsimd.dma_start(out=W["woT"][k],
                                        in_=woT_d[li, k * 128:(k + 1) * 128, :])
                W["dpd"] = w_dpd()
                for k in range(DCL):
                    nc.gpsimd.dma_start(out=W["dpd"][:, k * 128:(k + 1) * 128],
                                        in_=dpd_d[li, k])

            # stage state passed between emit phases
            ST = [dict() for _ in range(nstg)]

            def _fetch_bc(s, j):
                """Broadcast-load B and C rows 4j..4j+3 -> [128, 4*CB] tiles.

                4 separate row-DMAs per tile so they spread across DMA
                queues (one 1MB broadcast would not keep up with the scan).
                """
                li, ci = stages[s]
                cl = chunks[ci]
                bb = sp.tile([128, CB4], BF16, tag="bb", bufs=2, name="bb")
                cc = sp.tile([128, CB4], BF16, tag="cc", bufs=2, name="cc")
                for k in range(NG):
                    nc.sync.dma_start(
                        out=bb[:, k * CB:k * CB + cl],
                        in_=_bcast_row(bcst_d[s, NG * j + k, 0:cl]))
                    nc.sync.dma_start(
                        out=cc[:, k * CB:k * CB + cl],
                        in_=_bcast_row(bcst_d[s, 16 + NG * j + k, 0:cl]))
                ST[s].setdefault("bcq", []).append((bb, cc))

            def _fetch_next(s, p, j):
                """Prefetch chain: fetch the group after (s, p, j)."""
                if j + 1 < NG:
                    _fetch_bc(s, j + 1)
                elif p == 0:
                    _fetch_bc(s, 0)
                elif s + 1 < nstg:
                    _fetch_bc(s + 1, 0)

            def emit_front_portion(s, fp):
                li, ci = stages[s]
                cl = chunks[ci]
                qs = _qsplit(cl)
                st = ST[s]
                if fp == 0:
                    if s == 0:
                        for k in range(KC):
                            nc.sync.dma_start(out=xt[k], in_=x0[k * 128:(k + 1) * 128, :])
                    if ci == 0:
                        emit_weight_load(li)
                    st["W"] = dict(W)
                    st["u"] = sp.tile([128, DCL * CB], BF16, tag="uall",
                                      bufs=2, name="uall")
                    st["dt"] = sp.tile([128, DCL * CB], BF16, tag="dtall",
                                       bufs=2, name="dtall")
                    st["urem"] = sp.tile([128, DCL * CB], BF16, tag="urem",
                                         bufs=1, name="urem")
                if fp in (0, 1):
                    # per-d pipeline: in_proj -> conv -> silu -> x_proj accum
                    ds = range(0, 4) if fp == 0 else range(4, DCF)
                    for d in ds:
                        xch = sp.tile([128, CB + 4], BF16, tag="xch", bufs=2,
                                      name="xch")
                        # conv halo: zeros at worker start, saved tails after
                        if ci == 0:
                            nc.vector.memset(xch[:, 0:4], 0.0)
                        else:
                            tl = ST[s - 1]["tails_tile"]
                            nc.vector.tensor_copy(out=xch[:, 1:4],
                                                  in_=tl[:, d * 4:d * 4 + 3])
                        qoff = 0
                        for q in qs:
                            ps = qp.tile([128, 512], F32, tag="big", bufs=2,
                                         name="big")
                            for k in range(KC):
                                nc.tensor.matmul(
                                    ps[:, :q],
                                    st["W"]["wix"][k][:, d * 128:(d + 1) * 128],
                                    xt[k][:, cstart[ci] + qoff:
                                          cstart[ci] + qoff + q],
                                    start=(k == 0), stop=(k == KC - 1))
                            nc.scalar.copy(out=xch[:, 4 + qoff:4 + qoff + q],
                                           in_=ps[:, :q])
                            qoff += q
                        if ci + 1 < nch:
                            if d == 0:
                                st["tails_tile"] = sp.tile(
                                    [128, DCF * 4], BF16, tag="ctail",
                                    bufs=2, name="ctail")
                            nc.vector.tensor_copy(
                                out=st["tails_tile"][:, d * 4:d * 4 + 3],
                                in_=xch[:, cl + 1:cl + 4])
                        if d < DCL:
                            udst = st["u"]
                            uoff = d * CB
                        else:
                            udst = st["urem"]
                            uoff = (d - DCL) * CB
                        qoff = 0
                        for q in qs:
                            ps = qp.tile([128, 512], F32, tag="big", bufs=2,
                                         name="big")
                            for k in range(D_CONV):
                                nc.tensor.matmul(
                                    ps[:, :q],
                                    st["W"]["convd"][:, (d * D_CONV + k) * 128:
                                               (d * D_CONV + k + 1) * 128],
                                    xch[:, 1 + k + qoff:1 + k + qoff + q],
                                    start=(k == 0), stop=(k == D_CONV - 1))
                            # Copy (+conv bias) shares the exp act-table set,
                            # so interleaving with the scan's dA exps causes
                            # no ACT_TABLE_LOADs; silu is applied in one
                            # batched block in fp1.
                            nc.scalar.copy(
                                out=udst[:, uoff + qoff:uoff + qoff + q],
                                in_=ps[:, :q])
                            qoff += q
                    if fp == 1:
                        # batched silu(x + conv_bias) over all of u
                        # (one act-table switch pair per stage)
                        for d in range(DCF):
                            t_ = st["u"] if d < DCL else st["urem"]
                            uo = (d % DCL) * CB
                            nc.scalar.activation(
                                out=t_[:, uo:uo + cl], in_=t_[:, uo:uo + cl],
                                func=AF.Silu, bias=st["W"]["cb"][:, d:d + 1])
                        # x_proj contraction over all 8 chunks
                        xp = [qp.tile([128, 512], F32, tag="aux", bufs=2,
                                      name="aux") for _ in range(len(qs))]
                        for d in range(DCF):
                            src_t = st["u"] if d < DCL else st["urem"]
                            uoff = (d % DCL) * CB
                            qoff = 0
                            for qi, q in enumerate(qs):
                                nc.tensor.matmul(
                                    xp[qi][0:64, :q], st["W"]["wxT"][d],
                                    src_t[:, uoff + qoff:uoff + qoff + q],
                                    start=(d == 0), stop=(d == DCF - 1))
                                qoff += q
                        xdbl = sp.tile([64, CB], BF16, tag="xdbl", bufs=1,
                                       name="xdbl")
                        st["xdbl"] = xdbl
                        qoff = 0
                        for qi, q in enumerate(qs):
                            nc.scalar.copy(out=xdbl[:, qoff:qoff + q],
                                           in_=xp[qi][0:64, :q])
                            qoff += q
                if fp == 2:
                    xdbl = st["xdbl"]
                    nc.sync.dma_start(out=bcst_d[s, :, 0:cl],
                                      in_=xdbl[32:64, 0:cl])
                    # dt-proj softplus: exp into dt slices, then one
                    # in-place ln(x+1) over the whole tile (one table pair)
                    for d in range(DCL):
                        qoff = 0
                        for q in qs:
                            ps = qp.tile([128, 512], F32, tag="aux", bufs=2,
                                         name="aux")
                            nc.tensor.matmul(ps[:, :q],
                                             st["W"]["wdtT"][:, d * 128:(d + 1) * 128],
                                             xdbl[0:DT_RANK, qoff:qoff + q],
                                             start=True, stop=True)
                            nc.scalar.activation(
                                out=st["dt"][:, d * CB + qoff:d * CB + qoff + q],
                                in_=ps[:, :q], func=AF.Exp,
                                bias=st["W"]["bdt"][:, d:d + 1])
                            qoff += q
                    nc.scalar.activation(out=st["dt"][:, :], in_=st["dt"][:, :],
                                         func=AF.Ln, bias=1.0)
                    mt = sp.tile([128, CB], BF16, tag="maskt", bufs=1,
                                 name="maskt")
                    st["maskt"] = mt
                    src = mask_d[0, cstart[ci]:cstart[ci] + cl]
                    nc.sync.dma_start(out=mt[:, 0:cl], in_=_bcast_row(src))
                if fp == 3:
                    # z branch: matmuls + Copy out of psum, batched silu
                    st["szp"] = sp.tile([128, DCL * CB], BF16, tag="szp",
                                        bufs=1, name="szp")
                    qoff = 0
                    for q in qs:
                        for mz in range(MCZ):
                            ps = qp.tile([128, 512], F32, tag="big", bufs=2,
                                         name="big")
                            for k in range(KC):
                                nc.tensor.matmul(
                                    ps[:, :q],
                                    st["W"]["wiz"][k][:, mz * 128:(mz + 1) * 128],
                                    xt[k][:, cstart[ci] + qoff:
                                          cstart[ci] + qoff + q],
                                    start=(k == 0), stop=(k == KC - 1))
                            nc.scalar.copy(
                                out=st["szp"][:, mz * CB + qoff:
                                              mz * CB + qoff + q],
                                in_=ps[:, :q])
                        qoff += q
                    nc.scalar.activation(out=st["szp"][:, :],
                                         in_=st["szp"][:, :], func=AF.Silu)

            daq = []     # pre-emitted first-unit da4 (exp pipelining)

            def emit_exps(s, p, j, dlp):
                li, ci = stages[s]
                cl = chunks[ci]
                st = ST[s]
                dl = 2 * p + dlp
                da4 = sp.tile([128, CB4], BF16, tag="da4", bufs=2,
                              name="da4")
                for k in range(NG):
                    n = NG * j + k
                    col = dl * D_STATE + n
                    nc.scalar.activation(
                        out=da4[:, k * CB:k * CB + cl],
                        in_=st["dt"][:, dl * CB:dl * CB + cl],
                        func=AF.Exp,
                        scale=st["W"]["acol"][:, col:col + 1])
                daq.append(da4)

            def emit_dtup_mask(s):
                """dtup + dt-masking; emittable at the tail of the previous
                stage's pass 1 so the first unit's exps can precede the
                out-proj scalar copies in the scalar queue."""
                st = ST[s]
                st["dtup"] = sp.tile([128, DCL * CB], BF16, tag="dtup",
                                     bufs=1, name="dtup")
                dta = st["dt"]
                # dtup before masking; then mask dt in place (all 4 dl at once)
                nc.vector.tensor_tensor(out=st["dtup"][:, :], in0=dta[:, :],
                                        in1=st["u"][:, :], op=ALU.mult)
                mrep = _sap(st["maskt"], 0, [[0, DCL], [1, CB]])
                dview = _sap(dta, 0, [[CB, DCL], [1, CB]])
                nc.vector.tensor_tensor(out=dview, in0=dview, in1=mrep,
                                        op=ALU.add)

            def emit_scan_prelude(s):
                st = ST[s]
                st["yg"] = sp.tile([128, DCL * CB], BF16, tag="ygall",
                                   bufs=1, name="ygall")
                if not st.get("pre_done"):
                    emit_dtup_mask(s)
                if s == 0:
                    _fetch_bc(0, 0)

            def emit_scan_pass(s, p):
                """Pass p handles dl in {2p, 2p+1}: 4 n-groups x 2 dl.

                Front portions of stage s+1 are interleaved: after (p0,j1)
                -> fp0, (p0,j3) -> fp1, (p1,j1) -> fp2, (p1,j3) -> fp3.
                fp2 writes bcst(s+1), which (p1,j3)'s prefetch of
                (s+1, group 0) needs; fp3's z-silu runs after pass-1 gate.
                """
                li, ci = stages[s]
                cl = chunks[ci]
                st = ST[s]
                yacc = [qp.tile([128, CB], F32, tag="yacc", bufs=2,
                                name="yacc") for _ in range(2)]

                def emit_gate(dlp):
                    dl = 2 * p + dlp
                    nc.vector.tensor_tensor(
                        out=st["yg"][:, dl * CB:dl * CB + cl],
                        in0=yacc[dlp][:, 0:cl],
                        in1=st["szp"][:, dl * CB:dl * CB + cl],
                        op=ALU.mult)
                for j in range(NG):
                    if p == 0 and j == 1 and s > 0:
                        # stage's own z branch: deferred past the boundary so
                        # the first scan units' exps aren't queued behind it
                        emit_front_portion(s, 3)
                    _fetch_next(s, p, j)
                    bb, cc = st["bcq"].pop(0)
                    for dlp in range(2):
                        dl = 2 * p + dlp
                        if p == 0 and j == 0 and dlp == 0 and daq:
                            da4 = daq.pop(0)
                        else:
                            emit_exps(s, p, j, dlp)
                            da4 = daq.pop(0)
                        if ci > 0:
                            tmpb = sp.tile([128, NG - 1], BF16, tag="tmpb",
                                           bufs=2, name="tmpb")
                            hset = _sap(htile, (NG * j + 1) * DCL + dl,
                                        [[DCL, NG - 1]])
                            nc.vector.tensor_tensor(
                                out=tmpb[:, :],
                                in0=_sap(da4, CB, [[CB, NG - 1]]),
                                in1=hset, op=ALU.mult)
                        xs4 = sp.tile([128, CB4], BF16, tag="sc4", bufs=4,
                                      name="xs4")
                        drep = _sap(st["dtup"], dl * CB, [[0, NG], [1, CB]])
                        nc.vector.tensor_tensor(
                            out=_sap(xs4, 0, [[CB, NG], [1, CB]]), in0=drep,
                            in1=_sap(bb, 0, [[CB, NG], [1, CB]]),
                            op=ALU.mult)
                        if ci > 0:
                            xsb = _sap(xs4, CB, [[CB, NG - 1]])
                            nc.vector.tensor_tensor(out=xsb, in0=xsb,
                                                    in1=tmpb[:, :],
                                                    op=ALU.add)
                        nc.vector.memset(_sap(da4, CB, [[CB, NG - 1]]), 0.0)
                        h4 = sp.tile([128, CB4], BF16, tag="sc4", bufs=4,
                                     name="h4")
                        init = (htile[:, NG * j * DCL + dl:
                                      NG * j * DCL + dl + 1]
                                if ci > 0 else 0.0)
                        nc.vector.tensor_tensor_scan(
                            out=h4[:, :], data0=da4[:, :], data1=xs4[:, :],
                            initial=init, op0=ALU.mult, op1=ALU.add)
                        if ci + 1 < nch:
                            nc.vector.tensor_copy(
                                out=_sap(htile, NG * j * DCL + dl,
                                         [[DCL, NG]]),
                                in_=_sap(h4, CB - 1, [[CB, NG]]))
                        g4 = sp.tile([128, CB4], BF16, tag="sc4", bufs=4,
                                     name="g4")
                        nc.vector.tensor_tensor(out=g4[:, :], in0=h4[:, :],
                                                in1=cc[:, :], op=ALU.mult)
                        # y accumulation on PE: diag(Dp)*u first, then +g_n
                        qoff = 0
                        for q in _qsplit(cl):
                            if j == 0:
                                nc.tensor.matmul(
                                    yacc[dlp][:, qoff:qoff + q],
                                    st["W"]["dpd"][:, dl * 128:(dl + 1) * 128],
                                    st["u"][:, dl * CB + qoff:
                                            dl * CB + qoff + q],
                                    start=True, stop=False)
                            for k in range(NG):
                                nc.tensor.matmul(
                                    yacc[dlp][:, qoff:qoff + q],
                                    ident[:, :],
                                    g4[:, k * CB + qoff:k * CB + qoff + q],
                                    start=False,
                                    stop=(j == NG - 1 and k == NG - 1))
                            qoff += q
                        # gate dlp0 as soon as its accumulation closes: its
                        # PE drain then overlaps the final dlp1 unit
                        if j == NG - 1 and dlp == 0:
                            emit_gate(0)
                    if j == 1 and s + 1 < nstg:
                        emit_front_portion(s + 1, 2 * p)
                # dlp1 gate closes the pass (dlp0 was emitted in-loop)
                emit_gate(1)
                if p == 0 and s + 1 < nstg:
                    emit_front_portion(s + 1, 2 * p + 1)
                if p == 1 and s + 1 < nstg:
                    emit_dtup_mask(s + 1)
                    emit_exps(s + 1, 0, 0, 0)
                    ST[s + 1]["pre_done"] = True

            def emit_out(s):
                li, ci = stages[s]
                cl = chunks[ci]
                qs = _qsplit(cl)
                st = ST[s]
                qoff = 0
                for q in qs:
                    for mo in range(KC):
                        ps = qp.tile([128, 512], F32, tag="big", bufs=2,
                                     name="big")
                        for k in range(DCL):
                            nc.tensor.matmul(
                                ps[:, :q],
                                st["W"]["woT"][k][:, mo * 128:(mo + 1) * 128],
                                st["yg"][:, k * CB + qoff:k * CB + qoff + q],
                                start=(k == 0), stop=(k == DCL - 1))
                        ost = sp.tile([128, 512], BF16, tag="ost", bufs=2,
                                      name="ost")
                        nc.scalar.copy(out=ost[:, :q], in_=ps[:, :q])
                        if li + 1 < NL:
                            nc.sync.dma_start(
                                out=ccin_d[s][mo * 128:(mo + 1) * 128,
                                              qoff:qoff + q],
                                in_=ost[:, :q])
                        else:
                            nc.sync.dma_start(
                                out=x4[mo * 128:(mo + 1) * 128,
                                       cstart[ci] + qoff:cstart[ci] + qoff + q],
                                in_=ost[:, :q])
                    qoff += q
                if li + 1 < NL:
                    nc.gpsimd.collective_compute(
                        "AllReduce", ALU.add, replica_groups=RG,
                        ins=[ccin_d[s][:, :].opt()],
                        outs=[ccout_d[s][:, :].opt()])
                    for k in range(KC):
                        nc.sync.dma_start(
                            out=xt[k][:, cstart[ci]:cstart[ci] + cl],
                            in_=ccout_d[s][k * 128:(k + 1) * 128, :])

            # ---------------- schedule ----------------
            for fp in range(4):
                emit_front_portion(0, fp)
            for s in range(nstg):
                if s > 0:
                    emit_out(s - 1)
                emit_scan_prelude(s)
                emit_scan_pass(s, 0)
                emit_scan_pass(s, 1)
            emit_out(nstg - 1)
    nc.finalize()
    return nc


def _prep_host(inputs):
    inp = {k: np.asarray(v) for k, v in inputs.items()}
    rna = inp["rna_data_pad"].astype(np.int64)
    tis = inp["tissue_id"].astype(np.int64)
    lens = inp["seq_lengths"].astype(np.int64)
    T, chunks, pairs, offs = _plan(lens)

    x0 = inp["seq_emb"][rna] + inp["tissue_emb"][tis][:, None, :]
    x0 = x0 * (rna != 0)[..., None].astype(np.float32)
    x0T = np.ascontiguousarray(x0.transpose(0, 2, 1)).astype(np.float32)

    Wi = inp["W_in"].astype(np.float32)         # [NL, 2048, 512]
    cw = inp["conv_w"].astype(np.float32)       # [NL, 1024, 4]
    cbv = inp["conv_b"].astype(np.float32)      # [NL, 1024]
    Wx = inp["W_xproj"].astype(np.float32)      # [NL, 64, 1024]
    Wdt = inp["W_dt"].astype(np.float32)        # [NL, 1024, 32]
    bdtv = inp["b_dt"].astype(np.float32)
    A = -np.exp(inp["A_log"].astype(np.float64)).astype(np.float32)
    Dp = inp["D_par"].astype(np.float32)
    Wo = inp["W_out"].astype(np.float32)        # [NL, 512, 1024]

    idx = np.arange(128)
    ident = np.zeros((128, 128), np.float32)
    ident[idx, idx] = 1.0

    def cols(v, ndc):                            # [NL, ndc*128] -> [NL,128,ndc]
        return np.ascontiguousarray(
            v.reshape(NL, ndc, 128).transpose(0, 2, 1)).astype(np.float32)

    in_maps = []
    for w in range(4):
        a, b = pairs[w]
        xp = np.zeros((D_MODEL, T), np.float32)
        xp[:, 0:lens[a]] = x0T[a][:, 0:lens[a]]
        xp[:, offs[b]:offs[b] + lens[b]] = x0T[b][:, 0:lens[b]]
        xp = xp.astype(BF)
        mrow = np.zeros((1, T), np.float32)
        mrow[0, offs[b]] = MASK_HUGE
        mrow = mrow.astype(BF)
        for h_ in range(2):
            lo = h_ * 512
            perm = np.concatenate([np.arange(lo, lo + 512),
                                   np.arange((1 - h_) * 512, (1 - h_) * 512 + 512)])
            wix = np.ascontiguousarray(
                Wi[:, perm, :].transpose(0, 2, 1)).astype(BF)    # [NL,512,1024]
            wiz = np.ascontiguousarray(
                Wi[:, D_INNER + lo:D_INNER + lo + 512, :]
                .transpose(0, 2, 1)).astype(BF)                  # [NL,512,512]
            convd = np.zeros((NL, DCF, D_CONV, 128, 128), np.float32)
            cwp = cw[:, perm, :]
            for li in range(NL):
                for d in range(DCF):
                    for k in range(D_CONV):
                        convd[li, d, k, idx, idx] = cwp[li, d * 128:(d + 1) * 128, k]
            wxT = np.ascontiguousarray(
                Wx[:, :, perm].transpose(0, 2, 1)).astype(BF)    # [NL,1024,64]
            wdtT = np.ascontiguousarray(
                Wdt[:, lo:lo + 512, :].transpose(0, 2, 1)).astype(BF)  # [NL,32,512]
            woT = np.ascontiguousarray(
                Wo[:, :, lo:lo + 512].transpose(0, 2, 1)).astype(BF)   # [NL,512,512]
            acl = A[:, lo:lo + 512, :]                           # [NL,512,16]
            acol = np.ascontiguousarray(
                acl.reshape(NL, DCL, 128, D_STATE).transpose(0, 2, 1, 3)
                .reshape(NL, 128, DCL * D_STATE))
            dpd = np.zeros((NL, DCL, 128, 128), np.float32)
            dpl = Dp[:, lo:lo + 512]
            for li in range(NL):
                for d in range(DCL):
                    dpd[li, d, idx, idx] = dpl[li, d * 128:(d + 1) * 128]
            m = dict(
                x0=xp, maskrow=mrow, wix=wix, wiz=wiz,
                convd=convd.astype(BF), wxT=wxT, wdtT=wdtT, woT=woT,
                bdt=cols(bdtv[:, lo:lo + 512].reshape(NL, -1), DCL),
                cb=cols(cbv[:, perm].reshape(NL, -1), DCF),
                acol=acol.astype(np.float32),
                ident=ident.astype(BF),
                dpdiag=dpd.astype(BF),
            )
            in_maps.append(m)
    return inp, lens, T, chunks, pairs, offs, in_maps


def _head(inp, lens, pairs, offs, results):
    xw = []
    for w in range(4):
        xw.append(results[2 * w]["x4"].astype(np.float32) +
                  results[2 * w + 1]["x4"].astype(np.float32))
    outs = np.zeros((B, 1), np.float32)
    W1, b1 = inp["W1"].astype(np.float32), inp["b1"].astype(np.float32)
    W2, b2 = inp["W2"].astype(np.float32), inp["b2"].astype(np.float32)
    for w in range(4):
        for sidx in pairs[w]:
            x_last = xw[w][:, offs[sidx] + lens[sidx] - 1]
            h = np.maximum(x_last @ W1.T + b1, 0)
            outs[sidx] = h @ W2.T + b2
    return outs


def _run(inputs, trace=False):
    inp, lens, T, chunks, pairs, offs, in_maps = _prep_host(inputs)
    key = (T, chunks)
    if key not in _CACHE:
        _CACHE[key] = _build_nc(T, chunks)
    nc = _CACHE[key]
    kw = {}
    if trace:
        kw = dict(trace=True, trace_cores=[0])
    res = run_bass_kernel_spmd(nc, in_maps, core_ids=list(range(8)), **kw)
    out = _head(inp, lens, pairs, offs, res.results)
    return out, res


def kernel(**inputs) -> np.ndarray:
    out, _ = _run(inputs, trace=False)
    return out
